# revision 25
# baseline (speedup 1.0000x reference)
"""Fused linear + cross-entropy loss (Liger-style) on 8 TRN2 NeuronCores.

Problem: x[4096,4096] @ weight[32000,4096].T -> logits[4096,32000];
loss = mean_valid(logsumexp(logits) - logits[target]).

Sharding: vocab dim V=32000 split 8 ways (4000 rows/core, processed as
7 blocks of 512 + 1 block of 416).  Each core computes, for its vocab
shard, the per-token partial sum-exp (s_out) and the target logit if the
target index falls in its shard (t_out).  Host combines:
lse = log(sum of all partials), loss = sum((lse - tgt) * valid / n).

Logits are tiny (|z| < ~0.2: x,w ~ N(0, 0.02^2), H=4096), so the
max-subtraction in logsumexp is safely skipped on device.

Matmul runs in fp8(e4m3) with DoubleRow perf mode (2 fp8 weights/cell,
2 MACs/cycle): both x and w are scaled by 32 on host before the fp8 cast
(values ~N(0,0.64), well inside e4m3 range), so PSUM logits are 1024*z;
exp() descales via the activation scale param, the target-logit path is
descaled on host.

Input staging is the end-to-end bottleneck (host->HBM ~3.4GB/s), so the
kernel stages the minimum bytes (~148MB total): fp8 weights (one shard
per core, no replication, no padding), fp8 x staged as ONE 1/8
token-shard per core and reconstructed on device with an HBM AllGather,
a 16KB/core target row; iota is generated on device (gpsimd InstIota)
and the target match is computed as is_equal(iota - tgt, -vb*512).

Device layout: contraction h lands on SBUF partitions as [128k, 32ksub]
with h = ksub*128 + k; DoubleRow consumes adjacent ksub pairs.  The
weight shard (~15.6MB fp8) stays resident in SBUF; x streams per
512-token group ([128, 32, 512] fp8, one contiguous 2MB DMA each).
"""

import sys

for _p in ("/opt/trn_rl_repo",):
    if _p not in sys.path:
        sys.path.insert(0, _p)

from contextlib import ExitStack
from dataclasses import dataclass

import ml_dtypes
import numpy as np

import concourse.mybir as mybir
import concourse.tile as tile
from concourse import bacc
from concourse.bass_utils import run_bass_kernel_spmd

P = 128
IGNORE_INDEX = -100
SCALE = 32.0            # host-side scale on x and w before fp8 cast
ZSCALE = SCALE * SCALE  # psum logits = ZSCALE * true logits
N_CORES = 8
V_FULL = 32000


@dataclass
class Cfg:
    BT: int = 4096          # tokens
    H: int = 4096           # hidden
    VS: int = 4000          # vocab rows per core
    VBS: int = 512          # main vocab block (one PSUM bank)
    VBL: int = 416          # last vocab block (4000 = 7*512 + 416)
    GT: int = 512           # tokens per x-DMA group
    psum_bufs: int = 6

    @property
    def KSUB(self):
        return self.H // P          # 32 contraction subtiles

    @property
    def VB(self):
        return 8                    # vocab blocks (7x512 + 1x416)

    @property
    def widths(self):
        return [self.VBS] * 7 + [self.VBL]

    @property
    def BTILES(self):
        return self.BT // P          # 32 token tiles

    @property
    def NG(self):
        return self.BT // self.GT    # 8 groups

    @property
    def GTILES(self):
        return self.GT // P          # 4 token tiles per group


def build_nc(cfg: Cfg, allgather: bool = True):
    f32 = mybir.dt.float32
    f16 = mybir.dt.float16
    bf16 = mybir.dt.bfloat16
    f8 = mybir.dt.float8e4

    nc = bacc.Bacc("TRN2", target_bir_lowering=False, debug=False,
                   num_devices=N_CORES)
    wpm = nc.declare_dram_parameter(
        "wpm", [7, P, cfg.KSUB, cfg.VBS], f8, isOutput=False
    )
    wpl = nc.declare_dram_parameter(
        "wpl", [P, cfg.KSUB, cfg.VBL], f8, isOutput=False
    )
    if allgather:
        # per-core x token-shard (group c); AllGather reconstructs the
        # full x on device so we only stage 1/8 of x per core
        xsh = nc.declare_dram_parameter(
            "xsh", [P, cfg.KSUB, cfg.GT], f8, isOutput=False
        )
        xloc = nc.dram_tensor("xloc", [P, cfg.KSUB, cfg.GT], f8)
        xp = nc.dram_tensor(
            "xfull_shared", [cfg.NG, P, cfg.KSUB, cfg.GT], f8,
            addr_space="Shared"
        )
    else:
        # fallback: stage the full (replicated) x per core, no collective
        xp = nc.declare_dram_parameter(
            "xp", [cfg.NG, P, cfg.KSUB, cfg.GT], f8, isOutput=False
        )
    # tgt[p, j] = clipped target of token j*128+p, minus this core's v0
    tgt = nc.declare_dram_parameter("tgt", [P, cfg.BTILES], f32,
                                    isOutput=False)
    s_out = nc.declare_dram_parameter("s_out", [P, cfg.BTILES], f32,
                                      isOutput=True)
    t_out = nc.declare_dram_parameter("t_out", [P, cfg.BTILES], f32,
                                      isOutput=True)

    DR = mybir.MatmulPerfMode.DoubleRow

    with ExitStack() as ctx:
        tc = ctx.enter_context(tile.TileContext(nc))
        singles = ctx.enter_context(tc.tile_pool(name="singles", bufs=1))
        wpool = ctx.enter_context(tc.tile_pool(name="wpool", bufs=1))
        xpool = ctx.enter_context(tc.tile_pool(name="xpool", bufs=2))
        psum = ctx.enter_context(
            tc.tile_pool(name="psum", bufs=cfg.psum_bufs, space="PSUM")
        )
        scratch = ctx.enter_context(tc.tile_pool(name="scratch", bufs=4))
        stats = ctx.enter_context(tc.tile_pool(name="stats", bufs=2))
        outp = ctx.enter_context(tc.tile_pool(name="outp", bufs=2))

        tgt_sb = singles.tile([P, cfg.BTILES], f32, tag="tgt")
        nc.sync.dma_start(out=tgt_sb, in_=tgt.ap())
        # generate iota [0..VBS) on device instead of staging it
        iota_i = singles.tile([P, cfg.VBS], mybir.dt.int32, tag="iotai")
        nc.gpsimd.iota(iota_i, pattern=[[1, cfg.VBS]], base=0,
                       channel_multiplier=0)
        iota_sb = singles.tile([P, cfg.VBS], f32, tag="iota")
        nc.vector.tensor_copy(iota_sb, iota_i)

        if allgather:
            # copy the IO x-shard to an Internal DRAM tensor with one
            # direct DRAM->DRAM DMA (collectives cannot read IO
            # tensors), then gather all 8 shards
            nc.sync.dma_start(out=xloc[:], in_=xsh.ap())
            nc.gpsimd.collective_compute(
                "AllGather",
                mybir.AluOpType.bypass,
                replica_groups=[[i for i in range(N_CORES)]],
                ins=[xloc[:]],
                outs=[xp[:]],
            )

        def xp_group(g):
            return xp[:][g] if allgather else xp.ap()[g]

        # first group's x before the weight chunks so the vb-0 matmuls
        # can start after ~4MB of DMA instead of the full 16MB
        xg0 = xpool.tile([P, cfg.KSUB, cfg.GT], f8, tag="xg", name="xg0")
        nc.sync.dma_start(out=xg0, in_=xp_group(0))

        wchunk = []
        for vb in range(cfg.VB):
            wt = wpool.tile([P, cfg.KSUB, cfg.widths[vb]], f8, tag=f"w{vb}",
                            name=f"w{vb}")
            nc.sync.dma_start(
                out=wt, in_=wpm.ap()[vb] if vb < 7 else wpl.ap()
            )
            wchunk.append(wt)

        for g in range(cfg.NG):
            if g == 0:
                xg = xg0
            else:
                xg = xpool.tile([P, cfg.KSUB, cfg.GT], f8, tag="xg",
                                name=f"xg{g}")
                nc.sync.dma_start(out=xg, in_=xp_group(g))

            s_tiles = [
                stats.tile([P, cfg.VB], f32, tag=f"s{j}", name=f"s{j}")
                for j in range(cfg.GTILES)
            ]
            tacc = [
                stats.tile([P, cfg.VB], f32, tag=f"ta{j}", name=f"ta{j}")
                for j in range(cfg.GTILES)
            ]
            sg_t = outp.tile([P, cfg.GTILES], f32, tag="sg")
            tg_t = outp.tile([P, cfg.GTILES], f32, tag="tg")

            for jt in range(cfg.GTILES):
                tile_idx = g * cfg.GTILES + jt
                for vb in range(cfg.VB):
                    W = cfg.widths[vb]
                    pt = psum.tile([P, W], f32, tag="pt")
                    for b in range(cfg.KSUB // 2):
                        nc.tensor.matmul(
                            pt,
                            lhsT=xg[:, 2 * b:2 * b + 2, jt * P:(jt + 1) * P],
                            rhs=wchunk[vb][:, 2 * b:2 * b + 2, :],
                            start=(b == 0),
                            stop=(b == cfg.KSUB // 2 - 1),
                            perf_mode=DR,
                        )
                    # sum(exp(logits)) for this v-block -> s_tiles[jt][:, vb]
                    e = scratch.tile([P, W], bf16, tag="e")
                    nc.scalar.activation(
                        e, pt, mybir.ActivationFunctionType.Exp,
                        scale=1.0 / ZSCALE,
                        accum_out=s_tiles[jt][:, vb:vb + 1],
                    )
                    # target logit: (iota - tgt == -vb*VBS) selects the
                    # target column within this vocab block   (x ZSCALE)
                    eq = scratch.tile([P, W], bf16, tag="eq")
                    nc.vector.tensor_scalar(
                        eq, iota_sb[:, :W],
                        tgt_sb[:, tile_idx:tile_idx + 1],
                        float(-(vb * cfg.VBS)),
                        op0=mybir.AluOpType.subtract,
                        op1=mybir.AluOpType.is_equal,
                    )
                    sel = scratch.tile([P, W], f32, tag="sel")
                    nc.vector.tensor_tensor(
                        out=sel, in0=eq, in1=pt, op=mybir.AluOpType.mult
                    )
                    nc.vector.reduce_sum(
                        out=tacc[jt][:, vb:vb + 1], in_=sel,
                        axis=mybir.AxisListType.X,
                    )
                nc.vector.reduce_sum(
                    out=sg_t[:, jt:jt + 1], in_=s_tiles[jt],
                    axis=mybir.AxisListType.X,
                )
                nc.vector.reduce_sum(
                    out=tg_t[:, jt:jt + 1], in_=tacc[jt],
                    axis=mybir.AxisListType.X,
                )
            nc.sync.dma_start(
                out=s_out.ap()[:, g * cfg.GTILES:(g + 1) * cfg.GTILES],
                in_=sg_t,
            )
            nc.sync.dma_start(
                out=t_out.ap()[:, g * cfg.GTILES:(g + 1) * cfg.GTILES],
                in_=tg_t,
            )

    nc.compile()
    return nc


# ---------------------------------------------------------------- host side


def _prep_inputs(x, weight, target, cfg: Cfg, allgather: bool = True):
    f8 = ml_dtypes.float8_e4m3
    x = np.asarray(x, dtype=np.float32)
    weight = np.asarray(weight, dtype=np.float32)
    target = np.asarray(target).astype(np.int64)

    # x -> [NG, 128k, KSUB, GT] with h = ksub*128 + k
    xs = (x.T * SCALE).astype(f8)                       # [H, BT]
    xs = xs.reshape(cfg.KSUB, P, cfg.NG, cfg.GT)        # (ksub, k, g, t)
    xp = np.ascontiguousarray(xs.transpose(2, 1, 0, 3))  # [g, k, ksub, t]

    tgt_clip = np.clip(target, 0, V_FULL - 1)
    in_maps = []
    for c in range(N_CORES):
        v0 = c * cfg.VS
        ws = (weight[v0:v0 + cfg.VS].T * SCALE).astype(f8)  # [H, VS]
        ws = ws.reshape(cfg.KSUB, P, cfg.VS)            # (ksub, k, v)
        wpk = ws.transpose(1, 0, 2)                     # [k, ksub, v]
        wpm = np.ascontiguousarray(
            np.stack([wpk[:, :, i * cfg.VBS:(i + 1) * cfg.VBS]
                      for i in range(7)])
        )                                               # [7, k, ksub, 512]
        wpl = np.ascontiguousarray(wpk[:, :, 7 * cfg.VBS:])  # [k, ksub, 416]

        t_local = (tgt_clip - v0).astype(np.float32)
        tmat = np.ascontiguousarray(
            t_local.reshape(cfg.BTILES, P).T            # [P, BTILES]
        )
        m = {
            "wpm": wpm,
            "wpl": wpl,
            "tgt": tmat,
        }
        if allgather:
            m["xsh"] = np.ascontiguousarray(xp[c])
        else:
            m["xp"] = xp
        in_maps.append(m)
    return in_maps


def _combine(results, x, target, cfg: Cfg):
    target = np.asarray(target)
    s = np.stack([np.asarray(r["s_out"], dtype=np.float32) for r in results])
    t = np.stack([np.asarray(r["t_out"], dtype=np.float32) for r in results])
    sumexp = s.sum(axis=0).T.reshape(-1)                 # [BT], token = j*128+p
    lse = np.log(sumexp)
    tgt = t.sum(axis=0).T.reshape(-1) / ZSCALE           # [BT]
    valid = (target != IGNORE_INDEX)
    n = valid.sum()
    loss = ((lse - tgt) * valid / n).sum()
    return np.float32(loss)


def run(x, weight, target, cfg: Cfg | None = None, trace: bool = False,
        tmpdir: str | None = None, allgather: bool = True, **spmd_kwargs):
    cfg = cfg or Cfg()
    nc = build_nc(cfg, allgather=allgather)
    in_maps = _prep_inputs(x, weight, target, cfg, allgather=allgather)
    res = run_bass_kernel_spmd(
        nc, in_maps, list(range(N_CORES)), trace=trace, tmpdir=tmpdir,
        **spmd_kwargs,
    )
    loss = _combine(res.results, x, target, cfg)
    return loss, res


def kernel(x, weight, target):
    try:
        loss, _ = run(x, weight, target, allgather=True)
    except Exception:
        # fallback for environments where the AllGather NEFF doesn't
        # compile/run: stage the full (replicated) x per core instead
        loss, _ = run(x, weight, target, allgather=False)
    return loss


# revision 26
# speedup vs baseline: 1.0111x; 1.0111x over previous
"""Fused linear + cross-entropy loss (Liger-style) on 8 TRN2 NeuronCores.

Problem: x[4096,4096] @ weight[32000,4096].T -> logits[4096,32000];
loss = mean_valid(logsumexp(logits) - logits[target]).

Sharding: vocab dim V=32000 split 8 ways (4000 rows/core, processed as
7 blocks of 512 + 1 block of 416).  Each core computes, for its vocab
shard, the per-token partial sum-exp (s_out) and the target logit if the
target index falls in its shard (t_out).  Host combines:
lse = log(sum of all partials), loss = sum((lse - tgt) * valid / n).

Logits are tiny (|z| < ~0.2: x,w ~ N(0, 0.02^2), H=4096), so the
max-subtraction in logsumexp is safely skipped on device.

Matmul runs in fp8(e4m3) with DoubleRow perf mode (2 fp8 weights/cell,
2 MACs/cycle): both x and w are scaled by 32 on host before the fp8 cast
(values ~N(0,0.64), well inside e4m3 range), so PSUM logits are 1024*z;
exp() descales via the activation scale param, the target-logit path is
descaled on host.

Input staging is the end-to-end bottleneck (host->HBM ~3.4GB/s), so the
kernel stages the minimum bytes (~148MB total): fp8 weights (one shard
per core, no replication, no padding), fp8 x staged as ONE 1/8
token-shard per core and reconstructed on device with an HBM AllGather,
a 16KB/core target row; iota is generated on device (gpsimd InstIota)
and the target match is computed as is_equal(iota - tgt, -vb*512).

Device layout: contraction h lands on SBUF partitions as [128k, 32ksub]
with h = ksub*128 + k; DoubleRow consumes adjacent ksub pairs.  The
weight shard (~15.6MB fp8) stays resident in SBUF; x streams per
512-token group ([128, 32, 512] fp8, one contiguous 2MB DMA each).
"""

import sys

for _p in ("/opt/trn_rl_repo",):
    if _p not in sys.path:
        sys.path.insert(0, _p)

from contextlib import ExitStack
from dataclasses import dataclass

import ml_dtypes
import numpy as np

import concourse.mybir as mybir
import concourse.tile as tile
from concourse import bacc
from concourse.bass_utils import run_bass_kernel_spmd

P = 128
IGNORE_INDEX = -100
SCALE = 32.0            # host-side scale on x and w before fp8 cast
ZSCALE = SCALE * SCALE  # psum logits = ZSCALE * true logits
N_CORES = 8
V_FULL = 32000


@dataclass
class Cfg:
    BT: int = 4096          # tokens
    H: int = 4096           # hidden
    VS: int = 4000          # vocab rows per core
    VBS: int = 512          # main vocab block (one PSUM bank)
    VBL: int = 416          # last vocab block (4000 = 7*512 + 416)
    GT: int = 512           # tokens per x-DMA group
    psum_bufs: int = 8

    @property
    def KSUB(self):
        return self.H // P          # 32 contraction subtiles

    @property
    def VB(self):
        return 8                    # vocab blocks (7x512 + 1x416)

    @property
    def widths(self):
        return [self.VBS] * 7 + [self.VBL]

    @property
    def BTILES(self):
        return self.BT // P          # 32 token tiles

    @property
    def NG(self):
        return self.BT // self.GT    # 8 groups

    @property
    def GTILES(self):
        return self.GT // P          # 4 token tiles per group


def build_nc(cfg: Cfg, allgather: bool = True):
    f32 = mybir.dt.float32
    f16 = mybir.dt.float16
    bf16 = mybir.dt.bfloat16
    f8 = mybir.dt.float8e4

    nc = bacc.Bacc("TRN2", target_bir_lowering=False, debug=False,
                   num_devices=N_CORES)
    wpm = nc.declare_dram_parameter(
        "wpm", [7, P, cfg.KSUB, cfg.VBS], f8, isOutput=False
    )
    wpl = nc.declare_dram_parameter(
        "wpl", [P, cfg.KSUB, cfg.VBL], f8, isOutput=False
    )
    if allgather:
        # per-core x token-shard (group c); AllGather reconstructs the
        # full x on device so we only stage 1/8 of x per core
        xsh = nc.declare_dram_parameter(
            "xsh", [P, cfg.KSUB, cfg.GT], f8, isOutput=False
        )
        xloc = nc.dram_tensor("xloc", [P, cfg.KSUB, cfg.GT], f8)
        xp = nc.dram_tensor(
            "xfull_shared", [cfg.NG, P, cfg.KSUB, cfg.GT], f8,
            addr_space="Shared"
        )
    else:
        # fallback: stage the full (replicated) x per core, no collective
        xp = nc.declare_dram_parameter(
            "xp", [cfg.NG, P, cfg.KSUB, cfg.GT], f8, isOutput=False
        )
    # tgt[p, j] = clipped target of token j*128+p, minus this core's v0
    tgt = nc.declare_dram_parameter("tgt", [P, cfg.BTILES], f32,
                                    isOutput=False)
    s_out = nc.declare_dram_parameter("s_out", [P, cfg.BTILES], f32,
                                      isOutput=True)
    t_out = nc.declare_dram_parameter("t_out", [P, cfg.BTILES], f32,
                                      isOutput=True)

    DR = mybir.MatmulPerfMode.DoubleRow

    with ExitStack() as ctx:
        tc = ctx.enter_context(tile.TileContext(nc))
        singles = ctx.enter_context(tc.tile_pool(name="singles", bufs=1))
        wpool = ctx.enter_context(tc.tile_pool(name="wpool", bufs=1))
        xpool = ctx.enter_context(tc.tile_pool(name="xpool", bufs=2))
        psum = ctx.enter_context(
            tc.tile_pool(name="psum", bufs=cfg.psum_bufs, space="PSUM")
        )
        scratch = ctx.enter_context(tc.tile_pool(name="scratch", bufs=4))
        stats = ctx.enter_context(tc.tile_pool(name="stats", bufs=2))
        outp = ctx.enter_context(tc.tile_pool(name="outp", bufs=2))

        tgt_sb = singles.tile([P, cfg.BTILES], f32, tag="tgt")
        nc.sync.dma_start(out=tgt_sb, in_=tgt.ap())
        # generate iota [0..VBS) on device instead of staging it
        iota_i = singles.tile([P, cfg.VBS], mybir.dt.int32, tag="iotai")
        nc.gpsimd.iota(iota_i, pattern=[[1, cfg.VBS]], base=0,
                       channel_multiplier=0)
        iota_sb = singles.tile([P, cfg.VBS], f32, tag="iota")
        nc.vector.tensor_copy(iota_sb, iota_i)

        if allgather:
            # bounce the IO x-shard through SBUF into an Internal DRAM
            # tensor (collectives cannot read IO tensors), then gather
            # all 8 shards
            tin = singles.tile([P, cfg.KSUB, cfg.GT], f8, tag="xbounce")
            nc.sync.dma_start(out=tin, in_=xsh.ap())
            nc.sync.dma_start(out=xloc[:], in_=tin)
            nc.gpsimd.collective_compute(
                "AllGather",
                mybir.AluOpType.bypass,
                replica_groups=[[i for i in range(N_CORES)]],
                ins=[xloc[:]],
                outs=[xp[:]],
            )

        def xp_group(g):
            return xp[:][g] if allgather else xp.ap()[g]

        # first group's x before the weight chunks so the vb-0 matmuls
        # can start after ~4MB of DMA instead of the full 16MB
        xg0 = xpool.tile([P, cfg.KSUB, cfg.GT], f8, tag="xg", name="xg0")
        nc.sync.dma_start(out=xg0, in_=xp_group(0))

        wchunk = []
        for vb in range(cfg.VB):
            wt = wpool.tile([P, cfg.KSUB, cfg.widths[vb]], f8, tag=f"w{vb}",
                            name=f"w{vb}")
            nc.sync.dma_start(
                out=wt, in_=wpm.ap()[vb] if vb < 7 else wpl.ap()
            )
            wchunk.append(wt)

        for g in range(cfg.NG):
            if g == 0:
                xg = xg0
            else:
                xg = xpool.tile([P, cfg.KSUB, cfg.GT], f8, tag="xg",
                                name=f"xg{g}")
                nc.sync.dma_start(out=xg, in_=xp_group(g))

            s_tiles = [
                stats.tile([P, cfg.VB], f32, tag=f"s{j}", name=f"s{j}")
                for j in range(cfg.GTILES)
            ]
            tacc = [
                stats.tile([P, cfg.VB], f32, tag=f"ta{j}", name=f"ta{j}")
                for j in range(cfg.GTILES)
            ]
            sg_t = outp.tile([P, cfg.GTILES], f32, tag="sg")
            tg_t = outp.tile([P, cfg.GTILES], f32, tag="tg")

            for jt in range(cfg.GTILES):
                tile_idx = g * cfg.GTILES + jt
                for vb in range(cfg.VB):
                    W = cfg.widths[vb]
                    pt = psum.tile([P, W], f32, tag="pt")
                    for b in range(cfg.KSUB // 2):
                        nc.tensor.matmul(
                            pt,
                            lhsT=xg[:, 2 * b:2 * b + 2, jt * P:(jt + 1) * P],
                            rhs=wchunk[vb][:, 2 * b:2 * b + 2, :],
                            start=(b == 0),
                            stop=(b == cfg.KSUB // 2 - 1),
                            perf_mode=DR,
                        )
                    # sum(exp(logits)) for this v-block -> s_tiles[jt][:, vb]
                    e = scratch.tile([P, W], bf16, tag="e")
                    nc.scalar.activation(
                        e, pt, mybir.ActivationFunctionType.Exp,
                        scale=1.0 / ZSCALE,
                        accum_out=s_tiles[jt][:, vb:vb + 1],
                    )
                    # target logit: (iota - tgt == -vb*VBS) selects the
                    # target column within this vocab block   (x ZSCALE)
                    eq = scratch.tile([P, W], bf16, tag="eq")
                    nc.vector.tensor_scalar(
                        eq, iota_sb[:, :W],
                        tgt_sb[:, tile_idx:tile_idx + 1],
                        float(-(vb * cfg.VBS)),
                        op0=mybir.AluOpType.subtract,
                        op1=mybir.AluOpType.is_equal,
                    )
                    sel = scratch.tile([P, W], f32, tag="sel")
                    nc.vector.tensor_tensor(
                        out=sel, in0=eq, in1=pt, op=mybir.AluOpType.mult
                    )
                    nc.vector.reduce_sum(
                        out=tacc[jt][:, vb:vb + 1], in_=sel,
                        axis=mybir.AxisListType.X,
                    )
                nc.vector.reduce_sum(
                    out=sg_t[:, jt:jt + 1], in_=s_tiles[jt],
                    axis=mybir.AxisListType.X,
                )
                nc.vector.reduce_sum(
                    out=tg_t[:, jt:jt + 1], in_=tacc[jt],
                    axis=mybir.AxisListType.X,
                )
            nc.sync.dma_start(
                out=s_out.ap()[:, g * cfg.GTILES:(g + 1) * cfg.GTILES],
                in_=sg_t,
            )
            nc.sync.dma_start(
                out=t_out.ap()[:, g * cfg.GTILES:(g + 1) * cfg.GTILES],
                in_=tg_t,
            )

    nc.compile()
    return nc


# ---------------------------------------------------------------- host side


def _prep_inputs(x, weight, target, cfg: Cfg, allgather: bool = True):
    f8 = ml_dtypes.float8_e4m3
    x = np.asarray(x, dtype=np.float32)
    weight = np.asarray(weight, dtype=np.float32)
    target = np.asarray(target).astype(np.int64)

    # x -> [NG, 128k, KSUB, GT] with h = ksub*128 + k
    xs = (x.T * SCALE).astype(f8)                       # [H, BT]
    xs = xs.reshape(cfg.KSUB, P, cfg.NG, cfg.GT)        # (ksub, k, g, t)
    xp = np.ascontiguousarray(xs.transpose(2, 1, 0, 3))  # [g, k, ksub, t]

    tgt_clip = np.clip(target, 0, V_FULL - 1)
    in_maps = []
    for c in range(N_CORES):
        v0 = c * cfg.VS
        ws = (weight[v0:v0 + cfg.VS].T * SCALE).astype(f8)  # [H, VS]
        ws = ws.reshape(cfg.KSUB, P, cfg.VS)            # (ksub, k, v)
        wpk = ws.transpose(1, 0, 2)                     # [k, ksub, v]
        wpm = np.ascontiguousarray(
            np.stack([wpk[:, :, i * cfg.VBS:(i + 1) * cfg.VBS]
                      for i in range(7)])
        )                                               # [7, k, ksub, 512]
        wpl = np.ascontiguousarray(wpk[:, :, 7 * cfg.VBS:])  # [k, ksub, 416]

        t_local = (tgt_clip - v0).astype(np.float32)
        tmat = np.ascontiguousarray(
            t_local.reshape(cfg.BTILES, P).T            # [P, BTILES]
        )
        m = {
            "wpm": wpm,
            "wpl": wpl,
            "tgt": tmat,
        }
        if allgather:
            m["xsh"] = np.ascontiguousarray(xp[c])
        else:
            m["xp"] = xp
        in_maps.append(m)
    return in_maps


def _combine(results, x, target, cfg: Cfg):
    target = np.asarray(target)
    s = np.stack([np.asarray(r["s_out"], dtype=np.float32) for r in results])
    t = np.stack([np.asarray(r["t_out"], dtype=np.float32) for r in results])
    sumexp = s.sum(axis=0).T.reshape(-1)                 # [BT], token = j*128+p
    lse = np.log(sumexp)
    tgt = t.sum(axis=0).T.reshape(-1) / ZSCALE           # [BT]
    valid = (target != IGNORE_INDEX)
    n = valid.sum()
    loss = ((lse - tgt) * valid / n).sum()
    return np.float32(loss)


def run(x, weight, target, cfg: Cfg | None = None, trace: bool = False,
        tmpdir: str | None = None, allgather: bool = True, **spmd_kwargs):
    cfg = cfg or Cfg()
    nc = build_nc(cfg, allgather=allgather)
    in_maps = _prep_inputs(x, weight, target, cfg, allgather=allgather)
    res = run_bass_kernel_spmd(
        nc, in_maps, list(range(N_CORES)), trace=trace, tmpdir=tmpdir,
        **spmd_kwargs,
    )
    loss = _combine(res.results, x, target, cfg)
    return loss, res


def kernel(x, weight, target):
    try:
        loss, _ = run(x, weight, target, allgather=True)
    except Exception:
        # fallback for environments where the AllGather NEFF doesn't
        # compile/run: stage the full (replicated) x per core instead
        loss, _ = run(x, weight, target, allgather=False)
    return loss


# revision 27
# speedup vs baseline: 1.0317x; 1.0204x over previous
"""Fused linear + cross-entropy loss (Liger-style) on 8 TRN2 NeuronCores.

Problem: x[4096,4096] @ weight[32000,4096].T -> logits[4096,32000];
loss = mean_valid(logsumexp(logits) - logits[target]).

Sharding: vocab dim V=32000 split 8 ways (4000 rows/core, processed as
7 blocks of 512 + 1 block of 416).  Each core computes, for its vocab
shard, the per-token partial sum-exp (s_out) and the target logit if the
target index falls in its shard (t_out).  Host combines:
lse = log(sum of all partials), loss = sum((lse - tgt) * valid / n).

Logits are tiny (|z| < ~0.2: x,w ~ N(0, 0.02^2), H=4096), so the
max-subtraction in logsumexp is safely skipped on device.

Matmul runs in fp8(e4m3) with DoubleRow perf mode (2 fp8 weights/cell,
2 MACs/cycle): both x and w are scaled by 32 on host before the fp8 cast
(values ~N(0,0.64), well inside e4m3 range), so PSUM logits are 1024*z;
exp() descales via the activation scale param, the target-logit path is
descaled on host.

Input staging is the end-to-end bottleneck (host->HBM ~3.4GB/s), so the
kernel stages the minimum bytes (~148MB total): fp8 weights (one shard
per core, no replication, no padding), fp8 x staged as ONE 1/8
token-shard per core and reconstructed on device with an HBM AllGather,
a 16KB/core target row; iota is generated on device (gpsimd InstIota)
and the target match is computed as is_equal(iota - tgt, -vb*512).

Device layout: contraction h lands on SBUF partitions as [128k, 32ksub]
with h = ksub*128 + k; DoubleRow consumes adjacent ksub pairs.  The
weight shard (~15.6MB fp8) stays resident in SBUF; x streams per
512-token group ([128, 32, 512] fp8, one contiguous 2MB DMA each).
"""

import sys

for _p in ("/opt/trn_rl_repo",):
    if _p not in sys.path:
        sys.path.insert(0, _p)

from contextlib import ExitStack
from dataclasses import dataclass

import ml_dtypes
import numpy as np

import concourse.mybir as mybir
import concourse.tile as tile
from concourse import bacc
from concourse.bass_utils import run_bass_kernel_spmd

P = 128
IGNORE_INDEX = -100
SCALE = 32.0            # host-side scale on x and w before fp8 cast
ZSCALE = SCALE * SCALE  # psum logits = ZSCALE * true logits
N_CORES = 8
V_FULL = 32000


@dataclass
class Cfg:
    BT: int = 4096          # tokens
    H: int = 4096           # hidden
    VS: int = 4000          # vocab rows per core
    VBS: int = 512          # main vocab block (one PSUM bank)
    VBL: int = 416          # last vocab block (4000 = 7*512 + 416)
    GT: int = 512           # tokens per x-DMA group
    psum_bufs: int = 6

    @property
    def KSUB(self):
        return self.H // P          # 32 contraction subtiles

    @property
    def VB(self):
        return 8                    # vocab blocks (7x512 + 1x416)

    @property
    def widths(self):
        return [self.VBS] * 7 + [self.VBL]

    @property
    def BTILES(self):
        return self.BT // P          # 32 token tiles

    @property
    def NG(self):
        return self.BT // self.GT    # 8 groups

    @property
    def GTILES(self):
        return self.GT // P          # 4 token tiles per group


def build_nc(cfg: Cfg, allgather: bool = True):
    f32 = mybir.dt.float32
    f16 = mybir.dt.float16
    bf16 = mybir.dt.bfloat16
    f8 = mybir.dt.float8e4

    nc = bacc.Bacc("TRN2", target_bir_lowering=False, debug=False,
                   num_devices=N_CORES)
    wpm = nc.declare_dram_parameter(
        "wpm", [7, P, cfg.KSUB, cfg.VBS], f8, isOutput=False
    )
    wpl = nc.declare_dram_parameter(
        "wpl", [P, cfg.KSUB, cfg.VBL], f8, isOutput=False
    )
    if allgather:
        # per-core x token-shard (group c); AllGather reconstructs the
        # full x on device so we only stage 1/8 of x per core
        xsh = nc.declare_dram_parameter(
            "xsh", [P, cfg.KSUB, cfg.GT], f8, isOutput=False
        )
        xloc = nc.dram_tensor("xloc", [P, cfg.KSUB, cfg.GT], f8)
        xp = nc.dram_tensor(
            "xfull_shared", [cfg.NG, P, cfg.KSUB, cfg.GT], f8,
            addr_space="Shared"
        )
    else:
        # fallback: stage the full (replicated) x per core, no collective
        xp = nc.declare_dram_parameter(
            "xp", [cfg.NG, P, cfg.KSUB, cfg.GT], f8, isOutput=False
        )
    # tgt[p, j] = clipped target of token j*128+p, minus this core's v0
    tgt = nc.declare_dram_parameter("tgt", [P, cfg.BTILES], f32,
                                    isOutput=False)
    s_out = nc.declare_dram_parameter("s_out", [P, cfg.BTILES], f32,
                                      isOutput=True)
    t_out = nc.declare_dram_parameter("t_out", [P, cfg.BTILES], f32,
                                      isOutput=True)

    DR = mybir.MatmulPerfMode.DoubleRow

    with ExitStack() as ctx:
        tc = ctx.enter_context(tile.TileContext(nc))
        singles = ctx.enter_context(tc.tile_pool(name="singles", bufs=1))
        wpool = ctx.enter_context(tc.tile_pool(name="wpool", bufs=1))
        xpool = ctx.enter_context(tc.tile_pool(name="xpool", bufs=2))
        psum = ctx.enter_context(
            tc.tile_pool(name="psum", bufs=cfg.psum_bufs, space="PSUM")
        )
        scratch = ctx.enter_context(tc.tile_pool(name="scratch", bufs=4))
        stats = ctx.enter_context(tc.tile_pool(name="stats", bufs=2))
        outp = ctx.enter_context(tc.tile_pool(name="outp", bufs=2))

        tgt_sb = singles.tile([P, cfg.BTILES], f32, tag="tgt")
        nc.sync.dma_start(out=tgt_sb, in_=tgt.ap())
        # generate iota [0..VBS) on device instead of staging it
        iota_i = singles.tile([P, cfg.VBS], mybir.dt.int32, tag="iotai")
        nc.gpsimd.iota(iota_i, pattern=[[1, cfg.VBS]], base=0,
                       channel_multiplier=0)
        iota_sb = singles.tile([P, cfg.VBS], f32, tag="iota")
        nc.vector.tensor_copy(iota_sb, iota_i)

        if allgather:
            # bounce the IO x-shard through SBUF into an Internal DRAM
            # tensor (collectives cannot read IO tensors), then gather
            # all 8 shards
            tin = singles.tile([P, cfg.KSUB, cfg.GT], f8, tag="xbounce")
            nc.sync.dma_start(out=tin, in_=xsh.ap())
            nc.sync.dma_start(out=xloc[:], in_=tin)
            nc.gpsimd.collective_compute(
                "AllGather",
                mybir.AluOpType.bypass,
                replica_groups=[[i for i in range(N_CORES)]],
                ins=[xloc[:]],
                outs=[xp[:]],
            )

        def xp_group(g):
            return xp[:][g] if allgather else xp.ap()[g]

        # first group's x before the weight chunks so the vb-0 matmuls
        # can start after ~4MB of DMA instead of the full 16MB
        xg0 = xpool.tile([P, cfg.KSUB, cfg.GT], f8, tag="xg", name="xg0")
        nc.sync.dma_start(out=xg0, in_=xp_group(0))

        wchunk = []
        for vb in range(cfg.VB):
            wt = wpool.tile([P, cfg.KSUB, cfg.widths[vb]], f8, tag=f"w{vb}",
                            name=f"w{vb}")
            nc.sync.dma_start(
                out=wt, in_=wpm.ap()[vb] if vb < 7 else wpl.ap()
            )
            wchunk.append(wt)

        for g in range(cfg.NG):
            if g == 0:
                xg = xg0
            else:
                xg = xpool.tile([P, cfg.KSUB, cfg.GT], f8, tag="xg",
                                name=f"xg{g}")
                nc.sync.dma_start(out=xg, in_=xp_group(g))

            s_tiles = [
                stats.tile([P, cfg.VB], f32, tag=f"s{j}", name=f"s{j}")
                for j in range(cfg.GTILES)
            ]
            tacc = [
                stats.tile([P, cfg.VB], f32, tag=f"ta{j}", name=f"ta{j}")
                for j in range(cfg.GTILES)
            ]
            sg_t = outp.tile([P, cfg.GTILES], f32, tag="sg")
            tg_t = outp.tile([P, cfg.GTILES], f32, tag="tg")

            for jt in range(cfg.GTILES):
                tile_idx = g * cfg.GTILES + jt
                for vb in range(cfg.VB):
                    W = cfg.widths[vb]
                    pt = psum.tile([P, W], f32, tag="pt")
                    for b in range(cfg.KSUB // 2):
                        nc.tensor.matmul(
                            pt,
                            lhsT=xg[:, 2 * b:2 * b + 2, jt * P:(jt + 1) * P],
                            rhs=wchunk[vb][:, 2 * b:2 * b + 2, :],
                            start=(b == 0),
                            stop=(b == cfg.KSUB // 2 - 1),
                            perf_mode=DR,
                        )
                    # sum(exp(logits)) for this v-block -> s_tiles[jt][:, vb]
                    e = scratch.tile([P, W], bf16, tag="e")
                    nc.scalar.activation(
                        e, pt, mybir.ActivationFunctionType.Exp,
                        scale=1.0 / ZSCALE,
                        accum_out=s_tiles[jt][:, vb:vb + 1],
                    )
                    # target logit: (iota - tgt == -vb*VBS) selects the
                    # target column within this vocab block   (x ZSCALE)
                    eq = scratch.tile([P, W], bf16, tag="eq")
                    nc.vector.tensor_scalar(
                        eq, iota_sb[:, :W],
                        tgt_sb[:, tile_idx:tile_idx + 1],
                        float(-(vb * cfg.VBS)),
                        op0=mybir.AluOpType.subtract,
                        op1=mybir.AluOpType.is_equal,
                    )
                    sel = scratch.tile([P, W], f32, tag="sel")
                    nc.vector.tensor_tensor(
                        out=sel, in0=eq, in1=pt, op=mybir.AluOpType.mult
                    )
                    nc.vector.reduce_sum(
                        out=tacc[jt][:, vb:vb + 1], in_=sel,
                        axis=mybir.AxisListType.X,
                    )
                nc.vector.reduce_sum(
                    out=sg_t[:, jt:jt + 1], in_=s_tiles[jt],
                    axis=mybir.AxisListType.X,
                )
                nc.vector.reduce_sum(
                    out=tg_t[:, jt:jt + 1], in_=tacc[jt],
                    axis=mybir.AxisListType.X,
                )
            nc.sync.dma_start(
                out=s_out.ap()[:, g * cfg.GTILES:(g + 1) * cfg.GTILES],
                in_=sg_t,
            )
            nc.sync.dma_start(
                out=t_out.ap()[:, g * cfg.GTILES:(g + 1) * cfg.GTILES],
                in_=tg_t,
            )

    nc.compile()
    return nc


# ---------------------------------------------------------------- host side


def _prep_inputs(x, weight, target, cfg: Cfg, allgather: bool = True):
    f8 = ml_dtypes.float8_e4m3
    x = np.asarray(x, dtype=np.float32)
    weight = np.asarray(weight, dtype=np.float32)
    target = np.asarray(target).astype(np.int64)

    # x -> [NG, 128k, KSUB, GT] with h = ksub*128 + k
    xs = (x.T * SCALE).astype(f8)                       # [H, BT]
    xs = xs.reshape(cfg.KSUB, P, cfg.NG, cfg.GT)        # (ksub, k, g, t)
    xp = np.ascontiguousarray(xs.transpose(2, 1, 0, 3))  # [g, k, ksub, t]

    tgt_clip = np.clip(target, 0, V_FULL - 1)
    in_maps = []
    for c in range(N_CORES):
        v0 = c * cfg.VS
        ws = (weight[v0:v0 + cfg.VS].T * SCALE).astype(f8)  # [H, VS]
        ws = ws.reshape(cfg.KSUB, P, cfg.VS)            # (ksub, k, v)
        wpk = ws.transpose(1, 0, 2)                     # [k, ksub, v]
        wpm = np.ascontiguousarray(
            np.stack([wpk[:, :, i * cfg.VBS:(i + 1) * cfg.VBS]
                      for i in range(7)])
        )                                               # [7, k, ksub, 512]
        wpl = np.ascontiguousarray(wpk[:, :, 7 * cfg.VBS:])  # [k, ksub, 416]

        t_local = (tgt_clip - v0).astype(np.float32)
        tmat = np.ascontiguousarray(
            t_local.reshape(cfg.BTILES, P).T            # [P, BTILES]
        )
        m = {
            "wpm": wpm,
            "wpl": wpl,
            "tgt": tmat,
        }
        if allgather:
            m["xsh"] = np.ascontiguousarray(xp[c])
        else:
            m["xp"] = xp
        in_maps.append(m)
    return in_maps


def _combine(results, x, target, cfg: Cfg):
    target = np.asarray(target)
    s = np.stack([np.asarray(r["s_out"], dtype=np.float32) for r in results])
    t = np.stack([np.asarray(r["t_out"], dtype=np.float32) for r in results])
    sumexp = s.sum(axis=0).T.reshape(-1)                 # [BT], token = j*128+p
    lse = np.log(sumexp)
    tgt = t.sum(axis=0).T.reshape(-1) / ZSCALE           # [BT]
    valid = (target != IGNORE_INDEX)
    n = valid.sum()
    loss = ((lse - tgt) * valid / n).sum()
    return np.float32(loss)


def run(x, weight, target, cfg: Cfg | None = None, trace: bool = False,
        tmpdir: str | None = None, allgather: bool = True, **spmd_kwargs):
    cfg = cfg or Cfg()
    nc = build_nc(cfg, allgather=allgather)
    in_maps = _prep_inputs(x, weight, target, cfg, allgather=allgather)
    res = run_bass_kernel_spmd(
        nc, in_maps, list(range(N_CORES)), trace=trace, tmpdir=tmpdir,
        **spmd_kwargs,
    )
    loss = _combine(res.results, x, target, cfg)
    return loss, res


def kernel(x, weight, target):
    try:
        loss, _ = run(x, weight, target, allgather=True)
    except Exception:
        # fallback for environments where the AllGather NEFF doesn't
        # compile/run: stage the full (replicated) x per core instead
        loss, _ = run(x, weight, target, allgather=False)
    return loss
